# revision 5
# baseline (speedup 1.0000x reference)
"""CRF loss (nn_ConditionalRandomField) Bass/Trainium2 kernel.

Strategy
--------
loss = sum_b (numerator[b] - log_partition[b])

- log_partition (the B*T*N^2 forward scan — 99.99% of FLOPs) runs on 8
  NeuronCores, data-parallel over the batch dim (32 sequences/core).
- The scan runs in exp-space: A_t = (exp(trans)^T_pe @ A_{t-1}) * exp(emit_t),
  one PE matmul chain per step (bf16 inputs, f32 PSUM accumulation), with a
  sum-renormalization every 8 steps to stay in fp32/bf16 dynamic range.
  Renorm bookkeeping is exact: the actually-applied factor r (f32) has its
  log stashed on-chip and added back at the end.
- Layout: tag dim N=256 on partitions (2 tiles of 128), batch on the free
  dim. Host pre-transposes inputs to (N, T, B_core) per core — pure layout
  glue so DMA loads are contiguous. exp() of inputs runs on-device (ACT).
- numerator is a tiny O(B*T) gather -- computed on host in numpy.
"""

import numpy as np

B, T, N = 256, 512, 256
START, STOP = 254, 255
NCORES = 8
BC = B // NCORES  # 32 sequences per core
K_RENORM = 8


def _build_program(t_steps=T, chunk=64):
    """Build + compile the single-core SPMD Bass program."""
    import concourse.bass as bass
    import concourse.tile as tile
    from concourse import bacc, mybir

    f32 = mybir.dt.float32
    bf16 = mybir.dt.bfloat16
    EXP = mybir.ActivationFunctionType.Exp
    LN = mybir.ActivationFunctionType.Ln

    n_chunks = (t_steps + chunk - 1) // chunk
    assert t_steps % chunk == 0 or n_chunks * chunk >= t_steps
    renorm_ts = [t for t in range(1, t_steps - 1) if t % K_RENORM == K_RENORM - 1]
    n_renorm = len(renorm_ts)
    assert n_renorm <= 64

    nc = bacc.Bacc("TRN2", target_bir_lowering=False, debug=False,
                   enable_asserts=False)

    # DRAM I/O. x is the per-core input shard pre-transposed on host to
    # (n_tile, 128, T, BC) f32; transT is transitions.T (contiguous);
    # startcol/stopcol are trans[:, START] / trans[STOP, :] as columns.
    x = nc.dram_tensor("x", [2, 128, t_steps, BC], f32, kind="ExternalInput").ap()
    transT = nc.dram_tensor("transT", [2, 128, 256], f32, kind="ExternalInput").ap()
    startcol = nc.dram_tensor("startcol", [2, 128, 1], f32, kind="ExternalInput").ap()
    stopcol = nc.dram_tensor("stopcol", [2, 128, 1], f32, kind="ExternalInput").ap()
    denom_out = nc.dram_tensor("denom", [1, BC], f32, kind="ExternalOutput").ap()

    with tile.TileContext(nc) as tc:
        with (
            tc.tile_pool(name="consts", bufs=1) as consts,
            tc.tile_pool(name="wstage", bufs=1) as wstage,
            tc.tile_pool(name="ebig", bufs=1) as ebig,
            tc.tile_pool(name="stg", bufs=3) as stg,
            tc.tile_pool(name="apool", bufs=3) as apool,
            tc.tile_pool(name="tmp", bufs=2) as tmpp,
            tc.tile_pool(name="fin", bufs=1) as fin,
            tc.tile_pool(name="ps", bufs=2, space="PSUM") as psp,
            tc.tile_pool(name="pssm", bufs=2, space="PSUM") as pssm,
            tc.tile_pool(name="psb", bufs=1, space="PSUM") as psbp,
        ):
            # ---- constants ----
            ones128_bf = consts.tile([128, 1], bf16)
            nc.vector.memset(ones128_bf, 1.0)
            ones128_f = consts.tile([128, 1], f32)
            nc.vector.memset(ones128_f, 1.0)
            ones_row = consts.tile([1, 128], f32)
            nc.vector.memset(ones_row, 1.0)
            stash = consts.tile([1, BC, 64], f32)
            nc.vector.memset(stash, 0.0)

            # ---- weights: exp(transT) as bf16, 2 k-tiles of [128, 256] ----
            wtiles = []
            for k in range(2):
                wstg = wstage.tile([128, 256], f32, tag=f"wstg{k}")
                nc.sync.dma_start(out=wstg, in_=transT[k])
                wt = consts.tile([128, 256], bf16, tag=f"w{k}")
                nc.scalar.activation(out=wt, in_=wstg, func=EXP)
                wtiles.append(wt)

            expstart, expstop = [], []
            for k in range(2):
                sstg = wstage.tile([128, 1], f32, tag=f"sstg{k}")
                nc.sync.dma_start(out=sstg, in_=startcol[k])
                es = consts.tile([128, 1], f32, tag=f"es{k}")
                nc.scalar.activation(out=es, in_=sstg, func=EXP)
                expstart.append(es)
                pstg = wstage.tile([128, 1], f32, tag=f"pstg{k}")
                nc.sync.dma_start(out=pstg, in_=stopcol[k])
                ep = consts.tile([128, 1], f32, tag=f"ep{k}")
                nc.scalar.activation(out=ep, in_=pstg, func=EXP)
                expstop.append(ep)

            # ---- stream inputs: DMA f32 chunk -> ACT exp -> bf16 E tiles ----
            echunks = [[None] * n_chunks for _ in range(2)]
            for c in range(n_chunks):
                t0 = c * chunk
                t1 = min(t0 + chunk, t_steps)
                for j in range(2):
                    s = stg.tile([128, t1 - t0, BC], f32, tag=f"stg{j}")
                    nc.sync.dma_start(out=s, in_=x[j, :, t0:t1, :])
                    e = ebig.tile([128, t1 - t0, BC], bf16, tag=f"e{j}c{c}")
                    nc.scalar.activation(out=e, in_=s, func=EXP)
                    echunks[j][c] = e

            def eslice(j, t):
                return echunks[j][t // chunk][:, t % chunk, :]

            # ---- scan ----
            a_prev = []
            for j in range(2):
                a0 = apool.tile([128, BC], bf16, tag=f"a{j}")
                nc.vector.tensor_scalar_mul(a0, eslice(j, 0), expstart[j])
                a_prev.append(a0)

            pending_bcast = None
            n_stashed = 0
            for t in range(1, t_steps):
                a_new = []
                for j in range(2):
                    ps = psp.tile([128, BC], f32, tag=f"ps{j}")
                    nc.tensor.matmul(ps, wtiles[0][:, j * 128:(j + 1) * 128],
                                     a_prev[0], start=True, stop=False)
                    nc.tensor.matmul(ps, wtiles[1][:, j * 128:(j + 1) * 128],
                                     a_prev[1], start=False, stop=True)
                    an = apool.tile([128, BC], bf16, tag=f"a{j}")
                    if pending_bcast is None:
                        nc.vector.tensor_mul(an, ps, eslice(j, t))
                    else:
                        tm = tmpp.tile([128, BC], f32, tag=f"tm{j}")
                        nc.vector.tensor_mul(tm, ps, eslice(j, t))
                        nc.vector.tensor_mul(an, tm, pending_bcast)
                    a_new.append(an)
                pending_bcast = None
                a_prev = a_new

                if t in renorm_ts:
                    pss = pssm.tile([1, BC], f32, tag="pss")
                    nc.tensor.matmul(pss, ones128_bf, a_prev[0],
                                     start=True, stop=False)
                    nc.tensor.matmul(pss, ones128_bf, a_prev[1],
                                     start=False, stop=True)
                    r = tmpp.tile([1, BC], f32, tag="recip")
                    nc.vector.reciprocal(r, pss)
                    pb = psbp.tile([128, BC], f32, tag="pb")
                    nc.tensor.matmul(pb, ones_row, r, start=True, stop=True)
                    nc.scalar.activation(
                        out=stash[0:1, :, n_stashed],
                        in_=pb[0:1, :], func=LN)
                    n_stashed += 1
                    pending_bcast = pb

            # ---- finale ----
            # one last renorm so the stop-weighted sum stays within ACT Ln's
            # valid input range (A_L alone reaches ~2^79)
            pss = pssm.tile([1, BC], f32, tag="pss")
            nc.tensor.matmul(pss, ones128_bf, a_prev[0], start=True, stop=False)
            nc.tensor.matmul(pss, ones128_bf, a_prev[1], start=False, stop=True)
            r = tmpp.tile([1, BC], f32, tag="recip")
            nc.vector.reciprocal(r, pss)
            pb = psbp.tile([128, BC], f32, tag="pb")
            nc.tensor.matmul(pb, ones_row, r, start=True, stop=True)
            nc.scalar.activation(out=stash[0:1, :, n_stashed],
                                 in_=pb[0:1, :], func=LN)
            n_stashed += 1

            astop = []
            for j in range(2):
                af1 = tmpp.tile([128, BC], f32, tag=f"tm{j}")
                nc.vector.tensor_scalar_mul(af1, a_prev[j], expstop[j])
                af = fin.tile([128, BC], f32, tag=f"astop{j}")
                nc.vector.tensor_mul(af, af1, pb)
                astop.append(af)
            psw = pssm.tile([1, BC], f32, tag="pss")
            nc.tensor.matmul(psw, ones128_f, astop[0], start=True, stop=False)
            nc.tensor.matmul(psw, ones128_f, astop[1], start=False, stop=True)
            logsum = fin.tile([1, BC], f32, tag="logsum")
            import concourse.mybir as _mybir
            nc.vector.reduce_sum(logsum, stash, axis=_mybir.AxisListType.X)
            lnw = fin.tile([1, BC], f32, tag="lnw")
            nc.scalar.activation(out=lnw, in_=psw, func=LN)
            dn = fin.tile([1, BC], f32, tag="dn")
            nc.vector.tensor_sub(dn, lnw, logsum)
            nc.sync.dma_start(out=denom_out, in_=dn)

    nc.compile()
    return nc


_PROG_CACHE = {}


def _get_program(t_steps=T, chunk=64):
    key = (t_steps, chunk)
    if key not in _PROG_CACHE:
        _PROG_CACHE[key] = _build_program(t_steps, chunk)
    return _PROG_CACHE[key]


def _host_numerator(inputs, transitions, tags, mask):
    fm = mask.astype(np.float32)
    score = transitions[tags[:, 0], START].astype(np.float32)
    trans_sc = transitions[tags[:, 1:], tags[:, :-1]] * fm[:, 1:]
    emit_sc = np.take_along_axis(
        inputs[:, :-1, :], tags[:, :-1, None], axis=2)[..., 0] * fm[:, :-1]
    score = score + trans_sc.sum(-1) + emit_sc.sum(-1)
    last_idx = (fm.sum(-1) - 1.0).astype(np.int32)
    last_tags = np.take_along_axis(tags, last_idx[:, None], axis=1)[:, 0]
    last_input = np.take_along_axis(
        inputs[:, -1, :], last_tags[:, None], axis=1)[:, 0]
    return score + transitions[STOP, last_tags] + last_input * fm[:, -1]


def _make_in_maps(inputs, transitions):
    xt = np.ascontiguousarray(inputs.transpose(2, 1, 0))  # (N, T, B) f32
    transT = np.ascontiguousarray(transitions.T).reshape(2, 128, 256)
    sc = np.ascontiguousarray(transitions[:, START]).reshape(2, 128, 1)
    st = np.ascontiguousarray(transitions[STOP, :]).reshape(2, 128, 1)
    in_maps = []
    for c in range(NCORES):
        xc = np.ascontiguousarray(
            xt[:, :, c * BC:(c + 1) * BC]).reshape(2, 128, xt.shape[1], BC)
        in_maps.append({"x": xc, "transT": transT,
                        "startcol": sc, "stopcol": st})
    return in_maps


def kernel(inputs, transitions, tags, mask, _trace=False):
    from concourse.bass_utils import run_bass_kernel_spmd

    inputs = np.asarray(inputs, dtype=np.float32)
    transitions = np.asarray(transitions, dtype=np.float32)
    tags = np.asarray(tags)
    mask = np.asarray(mask)

    nc = _get_program()
    in_maps = _make_in_maps(inputs, transitions)
    res = run_bass_kernel_spmd(nc, in_maps, list(range(NCORES)), trace=_trace)
    denoms = np.concatenate([r["denom"].reshape(-1) for r in res.results])

    num = _host_numerator(inputs, transitions, tags, mask)
    out = np.float32(np.sum(num.astype(np.float64) - denoms.astype(np.float64)))
    if _trace:
        return out, res
    return out
